# revision 1
# baseline (speedup 1.0000x reference)
"""MicroHeadAttention Trainium2 kernel (8-core SPMD, data-parallel over
(batch, row-chunk) pairs).

Shapes (hardcoded): x (2, 2048, 1024), weights (1024, 1024), biases (1024,).
EMBED=1024, 16 heads in 2 blocks (g) of 8 micro-heads, head_dim 64.

Decomposition: the reference's "scramble" is a raw row-major reshape, so the
attention head (b, g, m') consumes exactly rows x[b, 256m':256(m'+1)] and
weight columns [512g:512(g+1)], reshaped (256, 512) -> (2048, 64) with
scrambled position n' = 8*row + m (m = 64-channel sub-block).  16 (b, m')
row-chunks across 8 cores = 2 per core; each chunk has g=0,1 -> 4 heads/core.

Per-core dataflow:
  phase 1: V = x@Wv.T+bv (natural row-major), scrambled to (n', d) layout via
           a DRAM round-trip DMA (with a ones-column appended for the softmax
           denominator).  Q^T/K^T computed channels-on-partitions; the
           mandatory PSUM->SBUF bias copies write strided so qsc/ksc come out
           directly in scrambled (d, n') layout, g on partition halves, both
           row-pairs in one (128, 4096) tile so each copy is (64, 512).
  phase 2: per head, per 512-wide q block: S^T = k^T.T @ q^T (K=64 matmuls,
           g=0/g=1 at partition bases 0/64 -> concurrent PE row-groups);
           two consecutive 128-wide k blocks share one (128, 1024) PSUM tile
           so a single ACT exp covers both (amortizes the ~352-cycle ACT
           fixed cost).  Causal masks added only on the 2 diagonal block
           pairs; no max subtraction (|S| < ~3).  ctx^T accumulated as
           [v | ones].T @ P^T giving ctx rows 0..64 and denominator row 64.
  phase 3: out = ctx^T.T @ Wo^T + bo in natural row layout; ctx^T is stored
           (during the divide-by-denominator copies) in a (c, rc, m, r)
           layout whose out-proj lhsT slices are contiguous and span both g
           blocks on the full 128 partitions.
"""

import numpy as np

import concourse.bass as bass
import concourse.mybir as mybir
from concourse import bacc
from concourse.tile import TileContext
from concourse.bass_utils import run_bass_kernel_spmd

F32 = mybir.dt.float32
FR = mybir.dt.float32r  # full-rate fp32 matmul dtype (TF32-like rounding)
DT_MM = FR
NEG = -1e30
E = 1024
R = 512       # rows per core
RP = 256      # rows per pair
ALU = mybir.AluOpType
ACTF = mybir.ActivationFunctionType

_cache = {}


def _build(loop_n=None, parts="all"):
    nc = bacc.Bacc()
    xT_d = nc.dram_tensor("xT", (E, R), F32, kind="ExternalInput")
    wq_d = nc.dram_tensor("wqT", (E, E), F32, kind="ExternalInput")
    wk_d = nc.dram_tensor("wkT", (E, E), F32, kind="ExternalInput")
    wv_d = nc.dram_tensor("wvT", (E, E), F32, kind="ExternalInput")
    wo_d = nc.dram_tensor("woTre", (128, 8, E), F32, kind="ExternalInput")
    bq_d = nc.dram_tensor("bqT", (128, 8), F32, kind="ExternalInput")
    bk_d = nc.dram_tensor("bkT8", (128, 8), F32, kind="ExternalInput")
    bv_d = nc.dram_tensor("bvrow", (1, E), F32, kind="ExternalInput")
    bo_d = nc.dram_tensor("borow", (1, E), F32, kind="ExternalInput")
    out_d = nc.dram_tensor("out", (R, E), F32, kind="ExternalOutput")

    with TileContext(nc) as tc:
        def body():
            with (
                tc.tile_pool(name="persist", bufs=1) as pp,
                tc.tile_pool(name="pt", bufs=3) as ptp,
                tc.tile_pool(name="misc", bufs=2) as mp,
                tc.tile_pool(name="dram", bufs=1, space="DRAM") as dp,
            ):
                # ---- persistent tiles ----
                bqT = pp.tile([128, 8], F32, tag="bqT", name="bqT")
                bkT8 = pp.tile([128, 8], F32, tag="bkT8", name="bkT8")
                # masks[k, v, 512c + q] = 0 where k <= q - 128*(2v+c) else NEG
                masks = pp.tile([128, 2, 1024], F32, tag="masks", name="masks")
                qsc = pp.tile([128, 4096], DT_MM, tag="qsc", name="qsc")
                ksc = pp.tile([128, 4096], DT_MM, tag="ksc", name="ksc")
                vsc = [[pp.tile([128, 16, 65], DT_MM, tag=f"vsc{p}{g}", name=f"vsc{p}{g}")
                        for g in range(2)] for p in range(2)]
                ctxP = [pp.tile([128, 2, 8, 128], DT_MM, tag=f"ctxP{p}", name=f"ctxP{p}")
                        for p in range(2)]
                vtmp = dp.tile([2, 2, 2048, 64], DT_MM, tag="vtmp", name="vtmp")

                nc.sync.dma_start(bqT[:], bq_d[:])
                nc.sync.dma_start(bkT8[:], bk_d[:])
                for v in range(2):
                    for c in range(2):
                        m = masks[:, v, 512 * c:512 * (c + 1)]
                        nc.gpsimd.memset(m, 0.0)
                        nc.gpsimd.affine_select(
                            out=m, in_=m, compare_op=ALU.is_ge, fill=NEG,
                            base=-(128 * (2 * v + c)), pattern=[[1, 512]],
                            channel_multiplier=-1)
                ones16 = pp.tile([128, 16], F32, tag="ones16", name="ones16")
                nc.gpsimd.memset(ones16[:], 1.0)
                for p in range(2):
                    for g in range(2):
                        nc.vector.tensor_copy(vsc[p][g][:, :, 64], ones16[:])

                with tc.tile_pool(name="stage1", bufs=1) as s1p, \
                     tc.tile_pool(name="ps1", bufs=5, space="PSUM") as psp:
                    xt = s1p.tile([128, 8, R], DT_MM, tag="xt", name="xt")
                    wq = s1p.tile([128, 8, E], DT_MM, tag="wq", name="wq")
                    xT_v = xT_d.rearrange("(ko ki) r -> ki ko r", ki=128).bitcast(DT_MM)
                    wq_v = wq_d.rearrange("(ko ki) o -> ki ko o", ki=128).bitcast(DT_MM)
                    for ko in range(8):
                        nc.sync.dma_start(wq[:, ko], wq_v[:, ko])
                        nc.sync.dma_start(xt[:, ko], xT_v[:, ko])

                    def qk_proj(w_tile, bias_tile, scale, dst):
                        for t in range(8):
                            ps = psp.tile([128, 512], F32, tag="psA", name="psA")
                            for ki in range(8):
                                nc.tensor.matmul(
                                    ps[:], w_tile[:, ki, 128 * t:128 * (t + 1)],
                                    xt[:, ki, :], start=(ki == 0), stop=(ki == 7))
                            g, u = t // 4, t % 4
                            for mh in range(2):
                                mmv = 2 * u + mh
                                # one strided copy covers both row-pairs
                                dest = dst.rearrange("c (j m) -> c j m", m=8)[
                                    64 * g:64 * (g + 1), :, mmv]
                                nc.scalar.activation(
                                    dest, ps[64 * mh:64 * (mh + 1), :],
                                    ACTF.Identity,
                                    bias=bias_tile[64 * mh:64 * (mh + 1), t:t + 1],
                                    scale=scale)

                    with tc.tile_pool(name="stagev", bufs=1) as svp:
                        wv = svp.tile([128, 8, E], DT_MM, tag="wv", name="wv")
                        vnat = [svp.tile([128, 2, E], DT_MM, tag=f"vnat{p}", name=f"vnat{p}")
                                for p in range(2)]
                        bvr = svp.tile([1, E], F32, tag="bvr", name="bvr")
                        bv_bc = svp.tile([128, E], F32, tag="bvbc", name="bvbc")
                        nc.sync.dma_start(bvr[:], bv_d[:])
                        nc.gpsimd.partition_broadcast(bv_bc[:], bvr[:])
                        wv_v = wv_d.rearrange("(ko ki) o -> ki ko o", ki=128).bitcast(DT_MM)
                        for ko in range(8):
                            nc.sync.dma_start(wv[:, ko], wv_v[:, ko])

                        if parts == "dmaonly":
                            nc.sync.dma_start(
                                out_d.rearrange("(a r) o -> r a o", r=128).bitcast(DT_MM),
                                wv[:, 0:4, :].rearrange("c a o -> c a o"))
                            return
                        for rc in range(4):
                            p, half = rc // 2, rc % 2
                            for oc in range(2):
                                ps = psp.tile([128, 512], F32, tag="psA", name="psA")
                                for ki in range(8):
                                    nc.tensor.matmul(
                                        ps[:], xt[:, ki, 128 * rc:128 * (rc + 1)],
                                        wv[:, ki, 512 * oc:512 * (oc + 1)],
                                        start=(ki == 0), stop=(ki == 7))
                                nc.vector.tensor_tensor(
                                    vnat[p][:, half, 512 * oc:512 * (oc + 1)],
                                    ps[:], bv_bc[:, 512 * oc:512 * (oc + 1)], ALU.add)
                        for p in range(2):
                            for g in range(2):
                                src = vnat[p][:, :, 512 * g:512 * (g + 1)].rearrange(
                                    "r h (m d) -> r h m d", m=8)
                                dst = vtmp[p, g].rearrange(
                                    "(h r m) d -> r h m d", h=2, r=128, m=8)
                                nc.sync.dma_start(dst, src)

                        # Q projection overlaps with the V scramble DMAs
                        qk_proj(wq, bqT, 1.0, qsc)

                    # wv/vnat freed; wk reuses that space
                    with tc.tile_pool(name="stagek", bufs=1) as skp:
                        wk = skp.tile([128, 8, E], DT_MM, tag="wk", name="wk")
                        wk_v = wk_d.rearrange("(ko ki) o -> ki ko o", ki=128).bitcast(DT_MM)
                        for ko in range(8):
                            nc.sync.dma_start(wk[:, ko], wk_v[:, ko])
                        for p in range(2):
                            for g in range(2):
                                nc.sync.dma_start(
                                    vsc[p][g][:, :, 0:64],
                                    vtmp[p, g].rearrange("(kb pin) d -> pin kb d", pin=128))
                        qk_proj(wk, bkT8, 0.125, ksc)

                # stage1 (xt, wq) freed; wo loads into that space
                with tc.tile_pool(name="stageo", bufs=1) as sop:
                    wo = sop.tile([128, 8, E], DT_MM, tag="wo", name="wo")
                    bor = sop.tile([1, E], F32, tag="bor", name="bor")
                    bo_bc = sop.tile([128, E], F32, tag="bobc", name="bobc")
                    nc.sync.dma_start(bor[:], bo_d[:])
                    nc.gpsimd.partition_broadcast(bo_bc[:], bor[:])
                    for ko in range(8):
                        nc.sync.dma_start(wo[:, ko], wo_d[:, ko].bitcast(DT_MM))

                    # ---- attention ----
                    if parts == "projonly":
                        nc.sync.dma_start(
                            out_d.rearrange("(a r) o -> r a o", r=128).bitcast(DT_MM),
                            qsc.rearrange("c (a o) -> c a o", a=4))
                        return
                    with tc.tile_pool(name="psS", bufs=2, space="PSUM") as pssp, \
                         tc.tile_pool(name="psctx", bufs=4, space="PSUM") as pcp:
                        for j5 in range(4):
                            for p in range(2):
                                nt2 = 2 * (j5 + 1)   # pairs of 128-wide k blocks
                                ctx_ps = [pcp.tile([65, 512], F32, tag="ctxps", name="ctxps")
                                          for _ in range(2)]
                                for t2 in range(nt2):
                                    st = [pssp.tile([128, 1024], F32, tag="st", name="st")
                                          for _ in range(2)]
                                    for half in range(2):
                                        kb = 2 * t2 + half
                                        for g in range(2):
                                            nc.tensor.matmul(
                                                st[g][:, 512 * half:512 * (half + 1)],
                                                ksc[64 * g:64 * (g + 1),
                                                    2048 * p + 128 * kb:2048 * p + 128 * (kb + 1)],
                                                qsc[64 * g:64 * (g + 1),
                                                    2048 * p + 512 * j5:2048 * p + 512 * (j5 + 1)],
                                                start=True, stop=True)
                                    pts = []
                                    for g in range(2):
                                        if t2 >= 2 * j5:      # diagonal pair
                                            nc.vector.tensor_tensor(
                                                st[g][:], st[g][:],
                                                masks[:, t2 - 2 * j5, :], ALU.add)
                                        pt = ptp.tile([128, 1024], DT_MM, tag="pt", name="pt")
                                        nc.scalar.activation(pt[:], st[g][:], ACTF.Exp)
                                        pts.append(pt)
                                    for half in range(2):
                                        kb = 2 * t2 + half
                                        for g in range(2):
                                            nc.tensor.matmul(
                                                ctx_ps[g][:], vsc[p][g][:, kb, :],
                                                pts[g][:, 512 * half:512 * (half + 1)],
                                                start=(kb == 0), stop=(kb == 4 * (j5 + 1) - 1))
                                for g in range(2):
                                    rec = mp.tile([1, 512], F32, tag="rec", name="rec")
                                    nc.vector.reciprocal(rec[:], ctx_ps[g][64:65, :])
                                    rbc = mp.tile([64, 512], F32, tag="rbc", name="rbc")
                                    nc.gpsimd.partition_broadcast(rbc[:], rec[:])
                                    dest = ctxP[p][64 * g:64 * (g + 1), j5 // 2, :,
                                                   64 * (j5 % 2):64 * (j5 % 2) + 64]
                                    nc.vector.tensor_tensor(
                                        dest,
                                        ctx_ps[g][0:64, :].rearrange("c (r m) -> c m r", m=8),
                                        rbc[:].rearrange("c (r m) -> c m r", m=8),
                                        ALU.mult)

                    # ---- output projection ----
                    if parts == "noout":
                        nc.sync.dma_start(
                            out_d.rearrange("(a r) o -> r a o", r=128).bitcast(DT_MM),
                            qsc.rearrange("c (a o) -> c a o", a=4))
                        return
                    with tc.tile_pool(name="psO", bufs=4, space="PSUM") as psop:
                        for p in range(2):
                            for rc in range(2):
                                for oc in range(2):
                                    ps = psop.tile([128, 512], F32, tag="psO", name="psO")
                                    for mmv in range(8):
                                        nc.tensor.matmul(
                                            ps[:],
                                            ctxP[p][:, rc, mmv, :],
                                            wo[:, mmv, 512 * oc:512 * (oc + 1)],
                                            start=(mmv == 0), stop=(mmv == 7))
                                    outsb = sop.tile([128, 512], F32, tag=f"outsb{p}{rc}{oc}", name="outsb")
                                    nc.vector.tensor_tensor(
                                        outsb[:], ps[:],
                                        bo_bc[:, 512 * oc:512 * (oc + 1)], ALU.add)
                                    nc.sync.dma_start(
                                        out_d[RP * p + 128 * rc:RP * p + 128 * (rc + 1),
                                              512 * oc:512 * (oc + 1)],
                                        outsb[:])

        if loop_n is None:
            body()
        else:
            with tc.For_i(0, loop_n, 1, hint_engines=(
                    mybir.EngineType.PE, mybir.EngineType.Activation,
                    mybir.EngineType.DVE, mybir.EngineType.SP,
                    mybir.EngineType.Pool)):
                body()
    nc.compile()
    return nc


def _get_nc(loop_n=None, parts="all"):
    key = ("nc", loop_n, parts)
    if key not in _cache:
        _cache[key] = _build(loop_n, parts)
    return _cache[key]


def kernel(x, Wq, bq, Wk, bk, Wv, bv, Wo, bo):
    x = np.asarray(x, np.float32)
    WqT = np.ascontiguousarray(np.asarray(Wq, np.float32).T)
    WkT = np.ascontiguousarray(np.asarray(Wk, np.float32).T)
    WvT = np.ascontiguousarray(np.asarray(Wv, np.float32).T)
    # woTre[64g + d, m, o] = Wo[o, 512g + 64m + d]
    WoTre = np.ascontiguousarray(
        np.asarray(Wo, np.float32).T.reshape(2, 8, 64, E).transpose(0, 2, 1, 3)
        .reshape(128, 8, E))
    bqT = np.ascontiguousarray(np.asarray(bq, np.float32).reshape(8, 128).T)
    bkT8 = np.ascontiguousarray((np.asarray(bk, np.float32) / 8.0).reshape(8, 128).T)
    bvrow = np.asarray(bv, np.float32).reshape(1, E)
    borow = np.asarray(bo, np.float32).reshape(1, E)

    in_maps = []
    for c in range(8):
        xTs = np.empty((E, R), np.float32)
        for p in range(2):
            h = 2 * c + p
            b_, mp_ = divmod(h, 8)
            xTs[:, RP * p:RP * (p + 1)] = x[b_, RP * mp_:RP * (mp_ + 1), :].T
        in_maps.append({
            "xT": np.ascontiguousarray(xTs), "wqT": WqT, "wkT": WkT,
            "wvT": WvT, "woTre": WoTre, "bqT": bqT, "bkT8": bkT8,
            "bvrow": bvrow, "borow": borow,
        })

    nc = _get_nc()
    res = run_bass_kernel_spmd(nc, in_maps, core_ids=list(range(8)))
    out = np.empty((2, 2048, E), np.float32)
    for c in range(8):
        o = res.results[c]["out"]
        for p in range(2):
            h = 2 * c + p
            b_, mp_ = divmod(h, 8)
            out[b_, RP * mp_:RP * (mp_ + 1), :] = o[RP * p:RP * (p + 1), :]
    return out



# revision 2
# speedup vs baseline: 349.1324x; 349.1324x over previous
"""MicroHeadAttention Trainium2 kernel v2 (8-core SPMD, data-parallel over
(batch, row-chunk) pairs).

Same decomposition as v1 (each core owns 4 heads: 2 row-pairs p x 2 g),
restructured for HW time:
  - all matmul operands bf16 (host pre-converts weights/x); PSUM stays f32.
  - consolidated DMAs, ordered xt -> wk -> wq -> wv -> (scrambles) -> wo.
  - Q/K projections split by row-pair p (N=256 moving operand) so
    attention for p=0 starts right after the p=0 projections; the p=1
    projections and V/out-proj work are interleaved into attention at
    j5 boundaries as PE filler.
  - Q/K bias copies on DVE (tensor_scalar), freeing ACT for the exp
    stream (the attention-phase pace-setter).
  - attention S tiles are one PSUM tile per 128-wide k block kb holding
    both head-groups [g0 512q | g1 512q]: one exp and (on the diagonal)
    one Pool affine_select (fill=0, post-exp) per kb, and true
    double-buffering within the 8 PSUM banks.
  - V scramble via DRAM round-trip in bf16, issued per row-chunk so
    vsc[p0] is ready before the first ctx matmul.
"""

import numpy as np

import concourse.bass as bass
import concourse.mybir as mybir
from concourse import bacc
from concourse.tile import TileContext
from concourse.bass_utils import run_bass_kernel_spmd

F32 = mybir.dt.float32
BF16 = mybir.dt.bfloat16
DT_MM = BF16
E = 1024
R = 512       # rows per core
RP = 256      # rows per pair
ALU = mybir.AluOpType
ACTF = mybir.ActivationFunctionType

_cache = {}


def _build(loop_n=None, parts="all"):
    nc = bacc.Bacc()
    xT_d = nc.dram_tensor("xT", (E, R), BF16, kind="ExternalInput")
    wq_d = nc.dram_tensor("wqT", (E, E), BF16, kind="ExternalInput")
    wk_d = nc.dram_tensor("wkT", (E, E), BF16, kind="ExternalInput")
    wv_d = nc.dram_tensor("wvT", (E, E), BF16, kind="ExternalInput")
    wo_d = nc.dram_tensor("woTre", (128, 8, E), BF16, kind="ExternalInput")
    bq_d = nc.dram_tensor("bqT", (128, 8), F32, kind="ExternalInput")
    bk_d = nc.dram_tensor("bkT8", (128, 8), F32, kind="ExternalInput")
    bv_d = nc.dram_tensor("bvrow", (1, E), F32, kind="ExternalInput")
    bo_d = nc.dram_tensor("borow", (1, E), F32, kind="ExternalInput")
    out_d = nc.dram_tensor("out", (R, E), BF16, kind="ExternalOutput")

    with TileContext(nc) as tc:
        def body():
            with (
                tc.tile_pool(name="persist", bufs=1) as pp,
                tc.tile_pool(name="pt", bufs=6) as ptp,
                tc.tile_pool(name="misc", bufs=2) as mp,
                tc.tile_pool(name="dram", bufs=1, space="DRAM") as dp,
            ):
                vtmp = dp.tile([2, 2, 2048, 64], DT_MM, tag="vtmp",
                               name="vtmp")
                # ---- persistent tiles ----
                bqT = pp.tile([128, 8], F32, tag="bqT", name="bqT")
                bkT8 = pp.tile([128, 8], F32, tag="bkT8", name="bkT8")
                qsc = [pp.tile([128, 2048], DT_MM, tag=f"qsc{p}",
                               name=f"qsc{p}") for p in range(2)]
                ksc = [pp.tile([128, 2048], DT_MM, tag=f"ksc{p}",
                               name=f"ksc{p}") for p in range(2)]
                vsc = [[pp.tile([128, 16, 65], DT_MM, tag=f"vsc{p}{g}",
                                name=f"vsc{p}{g}")
                        for g in range(2)] for p in range(2)]
                ctxP = [pp.tile([128, 2, 8, 128], DT_MM, tag=f"ctxP{p}",
                                name=f"ctxP{p}")
                        for p in range(2)]
                vnat = [pp.tile([128, 2, E], DT_MM, tag=f"vnat{p}",
                                name=f"vnat{p}")
                        for p in range(2)]
                xt = pp.tile([128, 8, R], DT_MM, tag="xt", name="xt")
                bvr = pp.tile([1, E], F32, tag="bvr", name="bvr")
                bv_bc = pp.tile([128, E], F32, tag="bvbc", name="bvbc")
                bor = pp.tile([1, E], F32, tag="bor", name="bor")
                bo_bc = pp.tile([128, E], F32, tag="bobc", name="bobc")
                wk = pp.tile([128, 8, E], DT_MM, tag="wk", name="wk")
                wq = pp.tile([128, 8, E], DT_MM, tag="wq", name="wq")
                wv = pp.tile([128, 8, E], DT_MM, tag="wv", name="wv")
                wo = pp.tile([128, 8, E], DT_MM, tag="wo", name="wo")

                # small loads off the bulk queue
                nc.scalar.dma_start(bqT[:], bq_d[:])
                nc.scalar.dma_start(bkT8[:], bk_d[:])
                nc.scalar.dma_start(bvr[:], bv_d[:])
                nc.scalar.dma_start(bor[:], bo_d[:])
                nc.gpsimd.partition_broadcast(bv_bc[:], bvr[:])
                nc.gpsimd.partition_broadcast(bo_bc[:], bor[:])
                for p in range(2):
                    for g in range(2):
                        nc.gpsimd.memset(vsc[p][g][:, :, 64], 1.0)

                # bulk DMA in consumption order; xt/wk split so the first
                # K-proj matmul can start after ~5us of loading
                xt_v = xT_d.rearrange("(ko ki) r -> ki ko r", ki=128)
                wk_v = wk_d.rearrange("(ko ki) o -> ki ko o", ki=128)
                nc.scalar.dma_start(xt[:, :, 0:RP], xt_v[:, :, 0:RP])
                nc.scalar.dma_start(xt[:, :, RP:R], xt_v[:, :, RP:R])
                for qtr in range(4):
                    nc.sync.dma_start(wk[:, :, 256 * qtr:256 * (qtr + 1)],
                                      wk_v[:, :, 256 * qtr:256 * (qtr + 1)])
                nc.sync.dma_start(
                    wq[:], wq_d.rearrange("(ko ki) o -> ki ko o", ki=128))
                nc.sync.dma_start(
                    wv[:], wv_d.rearrange("(ko ki) o -> ki ko o", ki=128))

                if parts == "dmaonly":
                    nc.sync.dma_start(wo[:], wo_d[:])
                    nc.sync.dma_start(
                        out_d.rearrange("(a r) o -> r a o", r=128),
                        wv[:, 0:4, :])
                    return

                psa_cm = tc.tile_pool(name="psA", bufs=2, space="PSUM")
                psa = psa_cm.__enter__()
                def psnext():
                    return psa.tile([128, 256], F32, tag="psA", name="psA")

                def qk_proj_half(w_tile, bias_tile, scale, dst, p, t):
                    # dst[64g+d, 8j+mmv] = scale*(x@W.T) + bias, rows of p
                    ps = psnext()
                    for ki in range(8):
                        nc.tensor.matmul(
                            ps[:], w_tile[:, ki, 128 * t:128 * (t + 1)],
                            xt[:, ki, RP * p:RP * (p + 1)],
                            start=(ki == 0), stop=(ki == 7))
                    g, u = t // 4, t % 4
                    for mh in range(2):
                        mmv = 2 * u + mh
                        dest = dst.rearrange("c (j m) -> c j m", m=8)[
                            64 * g:64 * (g + 1), :, mmv]
                        nc.vector.tensor_scalar(
                            dest, ps[64 * mh:64 * (mh + 1), :],
                            scale, bias_tile[64 * mh:64 * (mh + 1), t:t + 1],
                            ALU.mult, ALU.add)

                def v_group(rc):
                    # V projection for row-chunk rc + scramble DMAs
                    p, half = rc // 2, rc % 2
                    for oc in range(4):
                        ps = psnext()
                        for ki in range(8):
                            nc.tensor.matmul(
                                ps[:], xt[:, ki, 128 * rc:128 * (rc + 1)],
                                wv[:, ki, 256 * oc:256 * (oc + 1)],
                                start=(ki == 0), stop=(ki == 7))
                        nc.vector.tensor_tensor(
                            vnat[p][:, half, 256 * oc:256 * (oc + 1)],
                            ps[:], bv_bc[:, 256 * oc:256 * (oc + 1)],
                            ALU.add)
                    # vtmp[p, g, 1024 h + 8 r + m, d] = vnat[p][r, h, 512g+64m+d]
                    for g in range(2):
                        src = vnat[p][:, half, 512 * g:512 * (g + 1)] \
                            .rearrange("r (m d) -> r m d", m=8)
                        dst = vtmp[p, g].rearrange(
                            "(h r m) d -> h r m d", h=2, r=128, m=8)[half]
                        nc.sync.dma_start(dst, src)
                    for g in range(2):
                        nc.sync.dma_start(
                            vsc[p][g][:, 8 * half:8 * (half + 1), 0:64],
                            vtmp[p, g][1024 * half:1024 * (half + 1)]
                            .rearrange("(kb pin) d -> pin kb d", pin=128))

                # ---- p=0 projections ----
                for t in range(8):
                    qk_proj_half(wk, bkT8, 0.125, ksc[0], 0, t)
                for t in range(8):
                    qk_proj_half(wq, bqT, 1.0, qsc[0], 0, t)
                v_group(0)
                v_group(1)
                nc.sync.dma_start(wo[:], wo_d[:])

                if parts == "projonly":
                    for t in range(8):
                        qk_proj_half(wk, bkT8, 0.125, ksc[1], 1, t)
                    for t in range(8):
                        qk_proj_half(wq, bqT, 1.0, qsc[1], 1, t)
                    v_group(2)
                    v_group(3)
                    psa_cm.__exit__(None, None, None)
                    nc.sync.dma_start(
                        out_d.rearrange("(a r) o -> r a o", r=128),
                        qsc[0].rearrange("c (a o) -> c a o", a=2))
                    return  # noqa: B012

                # filler units interleaved into attention at j5 boundaries
                fill_p0 = (
                    [lambda t=t: qk_proj_half(wk, bkT8, 0.125, ksc[1], 1, t)
                     for t in range(8)]
                    + [lambda t=t: qk_proj_half(wq, bqT, 1.0, qsc[1], 1, t)
                       for t in range(8)]
                    + [lambda: v_group(2)]
                )
                fill_sched_p0 = [fill_p0[0:3], fill_p0[3:8],
                                 fill_p0[8:13], fill_p0[13:17]]
                fill_sched_p1 = [[lambda: v_group(3)], [], [], []]

                def attention(p, fill_sched, outproj_cb):
                    for j5 in range(4):
                        nkb = 4 * (j5 + 1)
                        ctx_ps = [pcp.tile([65, 512], F32, tag=f"ctxps{g}",
                                           name=f"ctxps{g}")
                                  for g in range(2)]
                        pts = [None] * nkb

                        def s_block(kb):
                            # columns q < off are fully masked: skip them in
                            # S / exp / ctx entirely
                            off = max(0, 128 * (kb - 4 * j5))
                            st = pssp.tile([128, 1024], F32, tag="st",
                                           name="st")
                            for g in range(2):
                                nc.tensor.matmul(
                                    st[:, 512 * g + off:512 * (g + 1)],
                                    ksc[p][64 * g:64 * (g + 1),
                                           128 * kb:128 * (kb + 1)],
                                    qsc[p][64 * g:64 * (g + 1),
                                           512 * j5 + off:512 * (j5 + 1)],
                                    start=True, stop=True)
                            pt = ptp.tile([128, 1024], DT_MM, tag="pt",
                                          name="pt")
                            nc.scalar.activation(
                                pt.rearrange("c (g q) -> c g q", g=2)[
                                    :, :, off:512],
                                st.rearrange("c (g q) -> c g q", g=2)[
                                    :, :, off:512],
                                ACTF.Exp)
                            if kb >= 4 * j5:  # diagonal block
                                # partially-masked window q in [off, off+128):
                                # keep where (q - off) - c >= 0, else 0
                                nc.gpsimd.affine_select(
                                    out=pt.rearrange("c (g q) -> c g q", g=2)[
                                        :, :, off:off + 128],
                                    in_=pt.rearrange("c (g q) -> c g q", g=2)[
                                        :, :, off:off + 128],
                                    compare_op=ALU.is_ge, fill=0.0,
                                    base=0, pattern=[[0, 2], [1, 128]],
                                    channel_multiplier=-1)
                            pts[kb] = (pt, off)

                        def ctx_block(kb):
                            pt, off = pts[kb]
                            for g in range(2):
                                nc.tensor.matmul(
                                    ctx_ps[g][:, off:512],
                                    vsc[p][g][:, kb, :],
                                    pt[:, 512 * g + off:512 * (g + 1)],
                                    start=(kb == 0), stop=(kb == nkb - 1))

                        # software pipeline: S(kb+1) issued before ctx(kb)
                        s_block(0)
                        for kb in range(nkb - 1):
                            s_block(kb + 1)
                            ctx_block(kb)
                        ctx_block(nkb - 1)
                        for g in range(2):
                            rec = mp.tile([1, 512], F32, tag="rec",
                                          name="rec")
                            nc.vector.reciprocal(rec[:], ctx_ps[g][64:65, :])
                            rbc = mp.tile([64, 512], F32, tag="rbc",
                                          name="rbc")
                            nc.gpsimd.partition_broadcast(rbc[:], rec[:])
                            dest = ctxP[p][64 * g:64 * (g + 1), j5 // 2, :,
                                           64 * (j5 % 2):64 * (j5 % 2) + 64]
                            nc.vector.tensor_tensor(
                                dest,
                                ctx_ps[g][0:64, :].rearrange(
                                    "c (r m) -> c m r", m=8),
                                rbc[:].rearrange("c (r m) -> c m r", m=8),
                                ALU.mult)
                        for f in fill_sched[j5]:
                            f()
                        if outproj_cb is not None:
                            outproj_cb(j5)

                def outproj_rc(p, rc):
                    # out-proj for one 128-row chunk, 4 x 256-wide psum
                    # accumulators (reuses the psA pool's single bank)
                    for oh in range(2):
                        outsb = mp.tile([128, 512], BF16, tag="outsb",
                                        name="outsb")
                        for oq in range(2):
                            oc = 2 * oh + oq
                            ps = psnext()
                            for mmv in range(8):
                                nc.tensor.matmul(
                                    ps[:], ctxP[p][:, rc, mmv, :],
                                    wo[:, mmv, 256 * oc:256 * (oc + 1)],
                                    start=(mmv == 0), stop=(mmv == 7))
                            nc.vector.tensor_tensor(
                                outsb[:, 256 * oq:256 * (oq + 1)], ps[:],
                                bo_bc[:, 256 * oc:256 * (oc + 1)], ALU.add)
                        nc.sync.dma_start(
                            out_d[RP * p + 128 * rc:RP * p + 128 * (rc + 1),
                                  512 * oh:512 * (oh + 1)],
                            outsb[:])

                with tc.tile_pool(name="psS", bufs=2, space="PSUM") as pssp, \
                     tc.tile_pool(name="psctx", bufs=1, space="PSUM") as pcp:
                    if parts == "noout":
                        attention(0, fill_sched_p0, None)
                        attention(1, fill_sched_p1, None)
                    else:
                        def op_cb0(j5):
                            if j5 == 2:
                                outproj_rc(0, 0)
                        def op_cb1(j5):
                            if j5 == 1:
                                outproj_rc(0, 1)
                            elif j5 == 2:
                                outproj_rc(1, 0)
                        attention(0, fill_sched_p0, op_cb0)
                        attention(1, fill_sched_p1, op_cb1)
                        outproj_rc(1, 1)
                psa_cm.__exit__(None, None, None)
                if parts == "noout":
                    nc.sync.dma_start(
                        out_d.rearrange("(a r) o -> r a o", r=128),
                        qsc[0].rearrange("c (a o) -> c a o", a=2))

        if loop_n is None:
            body()
        else:
            with tc.For_i(0, loop_n, 1, hint_engines=(
                    mybir.EngineType.PE, mybir.EngineType.Activation,
                    mybir.EngineType.DVE, mybir.EngineType.SP,
                    mybir.EngineType.Pool)):
                body()
    nc.compile()
    return nc


def _get_nc(loop_n=None, parts="all"):
    key = ("nc", loop_n, parts)
    if key not in _cache:
        _cache[key] = _build(loop_n, parts)
    return _cache[key]


def _bf16(a):
    import ml_dtypes
    return np.ascontiguousarray(a.astype(ml_dtypes.bfloat16))


def pack_inputs(x, Wq, bq, Wk, bk, Wv, bv, Wo, bo):
    x = np.asarray(x, np.float32)
    WqT = _bf16(np.asarray(Wq, np.float32).T)
    WkT = _bf16(np.asarray(Wk, np.float32).T)
    WvT = _bf16(np.asarray(Wv, np.float32).T)
    # woTre[64g + d, m, o] = Wo[o, 512g + 64m + d]
    WoTre = _bf16(
        np.asarray(Wo, np.float32).T.reshape(2, 8, 64, E).transpose(0, 2, 1, 3)
        .reshape(128, 8, E))
    bqT = np.ascontiguousarray(np.asarray(bq, np.float32).reshape(8, 128).T)
    bkT8 = np.ascontiguousarray(
        (np.asarray(bk, np.float32) / 8.0).reshape(8, 128).T)
    bvrow = np.asarray(bv, np.float32).reshape(1, E)
    borow = np.asarray(bo, np.float32).reshape(1, E)

    in_maps = []
    for c in range(8):
        xTs = np.empty((E, R), np.float32)
        for p in range(2):
            h = 2 * c + p
            b_, mp_ = divmod(h, 8)
            xTs[:, RP * p:RP * (p + 1)] = x[b_, RP * mp_:RP * (mp_ + 1), :].T
        in_maps.append({
            "xT": _bf16(xTs), "wqT": WqT, "wkT": WkT,
            "wvT": WvT, "woTre": WoTre, "bqT": bqT, "bkT8": bkT8,
            "bvrow": bvrow, "borow": borow,
        })
    return in_maps


def kernel(x, Wq, bq, Wk, bk, Wv, bv, Wo, bo):
    in_maps = pack_inputs(x, Wq, bq, Wk, bk, Wv, bv, Wo, bo)
    nc = _get_nc()
    res = run_bass_kernel_spmd(nc, in_maps, core_ids=list(range(8)))
    out = np.empty((2, 2048, E), np.float32)
    for c in range(8):
        o = np.asarray(res.results[c]["out"], dtype=np.float32)
        for p in range(2):
            h = 2 * c + p
            b_, mp_ = divmod(h, 8)
            out[b_, RP * mp_:RP * (mp_ + 1), :] = o[RP * p:RP * (p + 1), :]
    return out


# revision 8
# speedup vs baseline: 455.8718x; 1.3057x over previous
"""MicroHeadAttention Trainium2 kernel v2 (8-core SPMD, data-parallel over
(batch, row-chunk) pairs).

Same decomposition as v1 (each core owns 4 heads: 2 row-pairs p x 2 g),
restructured for HW time:
  - all matmul operands bf16 (host pre-converts weights/x); PSUM stays f32.
  - consolidated DMAs, ordered xt -> wk -> wq -> wv -> (scrambles) -> wo.
  - Q/K projections split by row-pair p (N=256 moving operand) so
    attention for p=0 starts right after the p=0 projections; the p=1
    projections and V/out-proj work are interleaved into attention at
    j5 boundaries as PE filler.
  - Q/K bias copies on DVE (tensor_scalar), freeing ACT for the exp
    stream (the attention-phase pace-setter).
  - attention S tiles are one PSUM tile per 128-wide k block kb holding
    both head-groups [g0 512q | g1 512q]: one exp and (on the diagonal)
    one Pool affine_select (fill=0, post-exp) per kb, and true
    double-buffering within the 8 PSUM banks.
  - V scramble via DRAM round-trip in bf16, issued per row-chunk so
    vsc[p0] is ready before the first ctx matmul.
"""

import numpy as np

import concourse.bass as bass
import concourse.mybir as mybir
from concourse import bacc
from concourse.tile import TileContext
from concourse.bass_utils import run_bass_kernel_spmd

F32 = mybir.dt.float32
BF16 = mybir.dt.bfloat16
DT_MM = BF16
E = 1024
R = 512       # rows per core
RP = 256      # rows per pair
ALU = mybir.AluOpType
ACTF = mybir.ActivationFunctionType

_cache = {}


def _build(loop_n=None, parts="all"):
    nc = bacc.Bacc()
    xT_d = nc.dram_tensor("xT", (E, R), BF16, kind="ExternalInput")
    wq_d = nc.dram_tensor("wqT", (E, E), BF16, kind="ExternalInput")
    wk_d = nc.dram_tensor("wkT", (E, E), BF16, kind="ExternalInput")
    wv_d = nc.dram_tensor("wvT", (E, E), BF16, kind="ExternalInput")
    wo_d = nc.dram_tensor("woTre", (128, 8, E), BF16, kind="ExternalInput")
    bq_d = nc.dram_tensor("bqT", (128, 8), F32, kind="ExternalInput")
    bk_d = nc.dram_tensor("bkT8", (128, 8), F32, kind="ExternalInput")
    bv_d = nc.dram_tensor("bvrow", (1, E), F32, kind="ExternalInput")
    bo_d = nc.dram_tensor("borow", (1, E), F32, kind="ExternalInput")
    out_d = nc.dram_tensor("out", (R, E), BF16, kind="ExternalOutput")

    with TileContext(nc) as tc:
        def body():
            with (
                tc.tile_pool(name="persist", bufs=1) as pp,
                tc.tile_pool(name="pt", bufs=6) as ptp,
                tc.tile_pool(name="misc", bufs=2) as mp,
                tc.tile_pool(name="dram", bufs=1, space="DRAM") as dp,
            ):
                vtmp = dp.tile([2, 2, 2048, 64], DT_MM, tag="vtmp",
                               name="vtmp")
                # ---- persistent tiles ----
                bqT = pp.tile([128, 8], F32, tag="bqT", name="bqT")
                bkT8 = pp.tile([128, 8], F32, tag="bkT8", name="bkT8")
                # q/k scrambled, stored [d-chan, n' = 8j+m]
                qsc = [pp.tile([128, 2048], DT_MM, tag=f"qsc{p}",
                               name=f"qsc{p}") for p in range(2)]
                ksc = [pp.tile([128, 2048], DT_MM, tag=f"ksc{p}",
                               name=f"ksc{p}") for p in range(2)]
                vsc = [[pp.tile([128, 16, 65], DT_MM, tag=f"vsc{p}{g}",
                                name=f"vsc{p}{g}")
                        for g in range(2)] for p in range(2)]
                # ctxP[c, rc, r128, m]: out-proj lhsT reads [:, rc, :, mmv]
                ctxP = [pp.tile([128, 2, 128, 8], DT_MM, tag=f"ctxP{p}",
                                name=f"ctxP{p}")
                        for p in range(2)]
                vnat = [pp.tile([128, 2, E], DT_MM, tag=f"vnat{p}",
                                name=f"vnat{p}")
                        for p in range(2)]
                xt = pp.tile([128, 8, R], DT_MM, tag="xt", name="xt")
                bvr = pp.tile([1, E], F32, tag="bvr", name="bvr")
                bv_bc = pp.tile([128, E], F32, tag="bvbc", name="bvbc")
                bor = pp.tile([1, E], F32, tag="bor", name="bor")
                bo_bc = pp.tile([128, E], F32, tag="bobc", name="bobc")
                wk = pp.tile([128, 8, E], DT_MM, tag="wk", name="wk")
                wq = pp.tile([128, 8, E], DT_MM, tag="wq", name="wq")
                wv = pp.tile([128, 8, E], DT_MM, tag="wv", name="wv")
                wo = pp.tile([128, 8, E], DT_MM, tag="wo", name="wo")

                # small loads off the bulk queue
                nc.scalar.dma_start(bqT[:], bq_d[:])
                nc.scalar.dma_start(bkT8[:], bk_d[:])
                nc.scalar.dma_start(bvr[:], bv_d[:])
                nc.scalar.dma_start(bor[:], bo_d[:])
                nc.gpsimd.partition_broadcast(bv_bc[:], bvr[:])
                nc.gpsimd.partition_broadcast(bo_bc[:], bor[:])
                for p in range(2):
                    for g in range(2):
                        nc.gpsimd.memset(vsc[p][g][:, :, 64], 1.0)

                # bulk DMA in consumption order; xt/wk split so the first
                # K-proj matmul can start after ~5us of loading
                xt_v = xT_d.rearrange("(ko ki) r -> ki ko r", ki=128)
                wk_v = wk_d.rearrange("(ko ki) o -> ki ko o", ki=128)
                nc.scalar.dma_start(xt[:, :, 0:RP], xt_v[:, :, 0:RP])
                nc.scalar.dma_start(xt[:, :, RP:R], xt_v[:, :, RP:R])
                for qtr in range(4):
                    nc.sync.dma_start(wk[:, :, 256 * qtr:256 * (qtr + 1)],
                                      wk_v[:, :, 256 * qtr:256 * (qtr + 1)])
                nc.sync.dma_start(
                    wq[:], wq_d.rearrange("(ko ki) o -> ki ko o", ki=128))
                nc.sync.dma_start(
                    wv[:], wv_d.rearrange("(ko ki) o -> ki ko o", ki=128))

                if parts == "dmaonly":
                    nc.sync.dma_start(wo[:], wo_d[:])
                    nc.sync.dma_start(
                        out_d.rearrange("(a r) o -> r a o", r=128),
                        wv[:, 0:4, :])
                    return

                psa_cm = tc.tile_pool(name="psA", bufs=2, space="PSUM")
                psa = psa_cm.__enter__()
                def psnext():
                    return psa.tile([128, 256], F32, tag="psA", name="psA")

                def qk_proj_half(w_tile, bias_tile, scale, dst, p, t,
                                 use_act=True):
                    # dst[64g+d, 8j+mmv] = scale*(x@W.T) + bias, rows of p
                    ps = psnext()
                    for ki in range(8):
                        nc.tensor.matmul(
                            ps[:], w_tile[:, ki, 128 * t:128 * (t + 1)],
                            xt[:, ki, RP * p:RP * (p + 1)],
                            start=(ki == 0), stop=(ki == 7))
                    g, u = t // 4, t % 4
                    for mh in range(2):
                        mmv = 2 * u + mh
                        dest = dst.rearrange("c (j m) -> c j m", m=8)[
                            64 * g:64 * (g + 1), :, mmv]
                        # the strided interleave writes are slow on every
                        # engine: split DVE/ACT so they run in parallel
                        # (ACT only while it has no exp work)
                        if mh == 1 and use_act:
                            nc.scalar.activation(
                                dest, ps[64:128, :], ACTF.Identity,
                                bias=bias_tile[64:128, t:t + 1], scale=scale)
                        else:
                            nc.vector.tensor_scalar(
                                dest, ps[64 * mh:64 * (mh + 1), :],
                                scale,
                                bias_tile[64 * mh:64 * (mh + 1), t:t + 1],
                                ALU.mult, ALU.add)

                def v_group(rc):
                    # V projection for row-chunk rc + scramble DMAs
                    p, half = rc // 2, rc % 2
                    for oc in range(4):
                        ps = psnext()
                        for ki in range(8):
                            nc.tensor.matmul(
                                ps[:], xt[:, ki, 128 * rc:128 * (rc + 1)],
                                wv[:, ki, 256 * oc:256 * (oc + 1)],
                                start=(ki == 0), stop=(ki == 7))
                        nc.vector.tensor_tensor(
                            vnat[p][:, half, 256 * oc:256 * (oc + 1)],
                            ps[:], bv_bc[:, 256 * oc:256 * (oc + 1)],
                            ALU.add)
                    # vtmp[p, g, 1024 h + 8 r + m, d] = vnat[p][r, h, 512g+64m+d]
                    for g in range(2):
                        src = vnat[p][:, half, 512 * g:512 * (g + 1)] \
                            .rearrange("r (m d) -> r m d", m=8)
                        dst = vtmp[p, g].rearrange(
                            "(h r m) d -> h r m d", h=2, r=128, m=8)[half]
                        nc.sync.dma_start(dst, src)
                    for g in range(2):
                        nc.sync.dma_start(
                            vsc[p][g][:, 8 * half:8 * (half + 1), 0:64],
                            vtmp[p, g][1024 * half:1024 * (half + 1)]
                            .rearrange("(kb pin) d -> pin kb d", pin=128))

                # ---- p=0 projections ----
                for t in range(8):
                    qk_proj_half(wk, bkT8, 0.125, ksc[0], 0, t)
                for t in range(8):
                    qk_proj_half(wq, bqT, 1.0, qsc[0], 0, t)
                v_group(0)
                v_group(1)
                nc.sync.dma_start(wo[:], wo_d[:])

                if parts == "projonly":
                    for t in range(8):
                        qk_proj_half(wk, bkT8, 0.125, ksc[1], 1, t)
                    for t in range(8):
                        qk_proj_half(wq, bqT, 1.0, qsc[1], 1, t)
                    v_group(2)
                    v_group(3)
                    psa_cm.__exit__(None, None, None)
                    nc.sync.dma_start(
                        out_d.rearrange("(a r) o -> r a o", r=128),
                        wv[:, 0:4, :])
                    return  # noqa: B012

                # filler units interleaved into attention at j5 boundaries
                fill_p0 = (
                    [lambda t=t: qk_proj_half(wk, bkT8, 0.125, ksc[1], 1, t,
                                              use_act=False)
                     for t in range(8)]
                    + [lambda t=t: qk_proj_half(wq, bqT, 1.0, qsc[1], 1, t,
                                                use_act=False)
                       for t in range(8)]
                    + [lambda: v_group(2)]
                )
                fill_sched_p0 = [fill_p0[0:3], fill_p0[3:8],
                                 fill_p0[8:13], fill_p0[13:17]]
                fill_sched_p1 = [[lambda: v_group(3)], [], [], []]

                def attention(p, fill_sched, outproj_cb):
                    for j5 in range(4):
                        nkb = 4 * (j5 + 1)
                        ctx_ps = [pcp.tile([65, 512], F32, tag=f"ctxps{g}",
                                           name=f"ctxps{g}")
                                  for g in range(2)]
                        pts = [None] * nkb

                        def s_block(kb):
                            # columns q < off are fully masked: skip them in
                            # S / exp / ctx entirely
                            off = max(0, 128 * (kb - 4 * j5))
                            st = pssp.tile([128, 1024], F32, tag="st",
                                           name="st")
                            for g in range(2):
                                nc.tensor.matmul(
                                    st[:, 512 * g + off:512 * (g + 1)],
                                    ksc[p][64 * g:64 * (g + 1),
                                           128 * kb:128 * (kb + 1)],
                                    qsc[p][64 * g:64 * (g + 1),
                                           512 * j5 + off:512 * (j5 + 1)],
                                    start=True, stop=True)
                            pt = ptp.tile([128, 1024], DT_MM, tag="pt",
                                          name="pt")
                            nc.scalar.activation(
                                pt.rearrange("c (g q) -> c g q", g=2)[
                                    :, :, off:512],
                                st.rearrange("c (g q) -> c g q", g=2)[
                                    :, :, off:512],
                                ACTF.Exp)
                            if kb >= 4 * j5:  # diagonal block
                                # partially-masked window q in [off, off+128):
                                # keep where (q - off) - c >= 0, else 0
                                nc.gpsimd.affine_select(
                                    out=pt.rearrange("c (g q) -> c g q", g=2)[
                                        :, :, off:off + 128],
                                    in_=pt.rearrange("c (g q) -> c g q", g=2)[
                                        :, :, off:off + 128],
                                    compare_op=ALU.is_ge, fill=0.0,
                                    base=0, pattern=[[0, 2], [1, 128]],
                                    channel_multiplier=-1)
                            pts[kb] = (pt, off)

                        def ctx_block(kb):
                            pt, off = pts[kb]
                            for g in range(2):
                                nc.tensor.matmul(
                                    ctx_ps[g][:, off:512],
                                    vsc[p][g][:, kb, :],
                                    pt[:, 512 * g + off:512 * (g + 1)],
                                    start=(kb == 0), stop=(kb == nkb - 1))

                        # software pipeline: S(kb+1) issued before ctx(kb)
                        s_block(0)
                        for kb in range(nkb - 1):
                            s_block(kb + 1)
                            ctx_block(kb)
                        ctx_block(nkb - 1)
                        for g in range(2):
                            rec = mp.tile([1, 512], F32, tag="rec",
                                          name="rec")
                            nc.vector.reciprocal(rec[:], ctx_ps[g][64:65, :])
                            rbc = mp.tile([64, 512], F32, tag="rbc",
                                          name="rbc")
                            nc.gpsimd.partition_broadcast(rbc[:], rec[:])
                            dest = ctxP[p][64 * g:64 * (g + 1), j5 // 2,
                                           64 * (j5 % 2):64 * (j5 % 2) + 64, :]
                            nc.vector.tensor_tensor(
                                dest.rearrange("c r m -> c (r m)"),
                                ctx_ps[g][0:64, :], rbc[:], ALU.mult)
                        for f in fill_sched[j5]:
                            f()
                        if outproj_cb is not None:
                            outproj_cb(j5)

                def outproj_rc(p, rc):
                    # out-proj for one 128-row chunk, 4 x 256-wide psum
                    # accumulators (reuses the psA pool's single bank)
                    for oh in range(2):
                        outsb = mp.tile([128, 512], BF16, tag="outsb",
                                        name="outsb")
                        for oq in range(2):
                            oc = 2 * oh + oq
                            ps = psnext()
                            for mmv in range(8):
                                nc.tensor.matmul(
                                    ps[:], ctxP[p][:, rc, :, mmv],
                                    wo[:, mmv, 256 * oc:256 * (oc + 1)],
                                    start=(mmv == 0), stop=(mmv == 7))
                            nc.vector.tensor_tensor(
                                outsb[:, 256 * oq:256 * (oq + 1)], ps[:],
                                bo_bc[:, 256 * oc:256 * (oc + 1)], ALU.add)
                        nc.sync.dma_start(
                            out_d[RP * p + 128 * rc:RP * p + 128 * (rc + 1),
                                  512 * oh:512 * (oh + 1)],
                            outsb[:])

                with tc.tile_pool(name="psS", bufs=2, space="PSUM") as pssp, \
                     tc.tile_pool(name="psctx", bufs=1, space="PSUM") as pcp:
                    if parts == "noout":
                        attention(0, fill_sched_p0, None)
                        attention(1, fill_sched_p1, None)
                    else:
                        def op_cb0(j5):
                            if j5 == 2:
                                outproj_rc(0, 0)
                        def op_cb1(j5):
                            if j5 == 1:
                                outproj_rc(0, 1)
                            elif j5 == 2:
                                outproj_rc(1, 0)
                        attention(0, fill_sched_p0, op_cb0)
                        attention(1, fill_sched_p1, op_cb1)
                        outproj_rc(1, 1)
                psa_cm.__exit__(None, None, None)
                if parts == "noout":
                    nc.sync.dma_start(
                        out_d.rearrange("(a r) o -> r a o", r=128),
                        wv[:, 0:4, :])

        if loop_n is None:
            body()
        else:
            with tc.For_i(0, loop_n, 1, hint_engines=(
                    mybir.EngineType.PE, mybir.EngineType.Activation,
                    mybir.EngineType.DVE, mybir.EngineType.SP,
                    mybir.EngineType.Pool)):
                body()
    nc.compile()
    return nc


def _get_nc(loop_n=None, parts="all"):
    key = ("nc", loop_n, parts)
    if key not in _cache:
        _cache[key] = _build(loop_n, parts)
    return _cache[key]


def _bf16(a):
    import ml_dtypes
    return np.ascontiguousarray(a.astype(ml_dtypes.bfloat16))


def pack_inputs(x, Wq, bq, Wk, bk, Wv, bv, Wo, bo):
    x = np.asarray(x, np.float32)
    WqT = _bf16(np.asarray(Wq, np.float32).T)
    WkT = _bf16(np.asarray(Wk, np.float32).T)
    WvT = _bf16(np.asarray(Wv, np.float32).T)
    # woTre[64g + d, m, o] = Wo[o, 512g + 64m + d]
    WoTre = _bf16(
        np.asarray(Wo, np.float32).T.reshape(2, 8, 64, E).transpose(0, 2, 1, 3)
        .reshape(128, 8, E))
    bqT = np.ascontiguousarray(np.asarray(bq, np.float32).reshape(8, 128).T)
    bkT8 = np.ascontiguousarray(
        (np.asarray(bk, np.float32) / 8.0).reshape(8, 128).T)
    bvrow = np.asarray(bv, np.float32).reshape(1, E)
    borow = np.asarray(bo, np.float32).reshape(1, E)

    in_maps = []
    for c in range(8):
        xTs = np.empty((E, R), np.float32)
        for p in range(2):
            h = 2 * c + p
            b_, mp_ = divmod(h, 8)
            xTs[:, RP * p:RP * (p + 1)] = x[b_, RP * mp_:RP * (mp_ + 1), :].T
        in_maps.append({
            "xT": _bf16(xTs), "wqT": WqT, "wkT": WkT,
            "wvT": WvT, "woTre": WoTre, "bqT": bqT, "bkT8": bkT8,
            "bvrow": bvrow, "borow": borow,
        })
    return in_maps


def kernel(x, Wq, bq, Wk, bk, Wv, bv, Wo, bo):
    in_maps = pack_inputs(x, Wq, bq, Wk, bk, Wv, bv, Wo, bo)
    nc = _get_nc()
    res = run_bass_kernel_spmd(nc, in_maps, core_ids=list(range(8)))
    out = np.empty((2, 2048, E), np.float32)
    for c in range(8):
        o = np.asarray(res.results[c]["out"], dtype=np.float32)
        for p in range(2):
            h = 2 * c + p
            b_, mp_ = divmod(h, 8)
            out[b_, RP * mp_:RP * (mp_ + 1), :] = o[RP * p:RP * (p + 1), :]
    return out
